# revision 1
# baseline (speedup 1.0000x reference)
"""Multi-head attention, tensor-parallel across 8 Trainium2 NeuronCores.

Sharding: core = (batch b, head-group g) with b in {0,1}, g in {0..3}.
Each core computes 4 heads (a 256-wide slice of the head dimension) for one
batch element:
  Q^T/K^T = Wq/Wk slice^T-projections of query/key (kept transposed: [dh, s])
  V       = value @ Wv slice (natural [s, dh]), with an appended ones column
  S^T     = K^T-chunk.T @ Q^T-chunk per head  -> scores transposed [j, i]
  E       = exp(S^T * scale)                  (no max subtraction; scores ~N(0,1))
  [O^T;Z] = V'.T @ E  accumulated over j      (ones column yields Z = sum_j E)
  Onorm^T = O^T * (1/Z) broadcast
  outT    = Wo-slice.T @ Onorm^T (+ bo on group-0 cores only)
Host: transposes activations into [D, S] per core, and sums the 4 group
partials per batch (the "all-reduce" of the output projection), then
transposes back.

Inputs arrive full-size; all sharding is internal.
"""

import numpy as np

# Problem shape (hardcoded per the harness contract).
B, S, D, H = 2, 2048, 1024, 16
DK = D // H              # 64 head dim
N_CORES = 8
GROUPS = N_CORES // B    # 4 head-groups
DH = D // GROUPS         # 256 head-dims per core (4 heads)
H_CORE = DH // DK        # 4 heads per core
SCALE = 1.0 / float(np.sqrt(DK))

P = 128                  # SBUF/PSUM partitions
SC = 512                 # matmul moving-dim chunk (one PSUM bank of fp32)
IB = 1024                # flash i-block (exp granule)


def build_nc(S=S, D=D, DH=DH, DK=DK, scale=SCALE, ib=IB, dtype="f32r"):
    """Build the per-core Bass module (same NEFF for all 8 cores)."""
    import concourse.bacc as bacc
    import concourse.mybir as mybir
    import concourse.tile as tile

    f32 = mybir.dt.float32
    f32r = mybir.dt.float32r
    bf16 = mybir.dt.bfloat16
    Exp = mybir.ActivationFunctionType.Exp

    KT = D // P                    # contraction tiles for projections
    NSC = S // SC                  # s chunks
    HC = DH // P                   # head-dim chunks (2)
    HPC = P // DK                  # heads per chunk (2)
    H_CORE = DH // DK
    JT = S // P                    # j tiles
    NIB = S // ib                  # i blocks
    ICB = ib // SC                 # i chunks per block
    NOUT = D // P                  # output row chunks

    cdt = {"f32r": f32r, "bf16": bf16, "f32": f32}[dtype]

    def mm(ap):
        return ap

    nc = bacc.Bacc("TRN2", target_bir_lowering=False, debug=False)

    qT = nc.dram_tensor("qT", [D, S], cdt, kind="ExternalInput")
    kTd = nc.dram_tensor("kTd", [D, S], cdt, kind="ExternalInput")
    vT = nc.dram_tensor("vT", [D, S], cdt, kind="ExternalInput")
    wq = nc.dram_tensor("wq", [D, DH], cdt, kind="ExternalInput")
    wk = nc.dram_tensor("wk", [D, DH], cdt, kind="ExternalInput")
    wv = nc.dram_tensor("wv", [D, DH], cdt, kind="ExternalInput")
    wo = nc.dram_tensor("wo", [DH, D], cdt, kind="ExternalInput")
    bq = nc.dram_tensor("bq", [P, HC], f32, kind="ExternalInput")
    bk = nc.dram_tensor("bk", [P, HC], f32, kind="ExternalInput")
    bvb = nc.dram_tensor("bvb", [P, H_CORE, DK], f32, kind="ExternalInput")
    bo = nc.dram_tensor("bo", [P, NOUT], f32, kind="ExternalInput")
    outT = nc.dram_tensor("outT", [D, S], f32, kind="ExternalOutput")

    with tile.TileContext(nc) as tc:
        with (
            tc.tile_pool(name="const", bufs=1) as cpool,
            tc.tile_pool(name="pers", bufs=1) as pers,
            tc.tile_pool(name="stream", bufs=1) as stream,
            tc.tile_pool(name="psum", bufs=1, space="PSUM") as psum,
            tc.tile_pool(name="dscratch", bufs=1, space="DRAM") as dscratch,
        ):
            # ---- constants ----
            wq_sb = cpool.tile([P, KT, DH], cdt, name="wq_sb")
            wk_sb = cpool.tile([P, KT, DH], cdt, name="wk_sb")
            wv_sb = cpool.tile([P, KT, DH], cdt, name="wv_sb")
            wo_sb = cpool.tile([P, HC, D], cdt, name="wo_sb")
            bq_sb = cpool.tile([P, HC], f32, name="bq_sb")
            bk_sb = cpool.tile([P, HC], f32, name="bk_sb")
            bvb_sb = cpool.tile([P, H_CORE, DK], f32, name="bvb_sb")
            bo_sb = cpool.tile([P, NOUT], f32, name="bo_sb")
            nc.sync.dma_start(wq_sb[:], qT_ap_rearr(wq, P))
            nc.sync.dma_start(wk_sb[:], qT_ap_rearr(wk, P))
            nc.sync.dma_start(wv_sb[:], qT_ap_rearr(wv, P))
            nc.sync.dma_start(wo_sb[:], wo[:, :].rearrange("(c p) n -> p c n", p=P))
            nc.sync.dma_start(bq_sb[:], bq[:, :])
            nc.sync.dma_start(bk_sb[:], bk[:, :])
            nc.sync.dma_start(bvb_sb[:], bvb[:, :, :])
            nc.sync.dma_start(bo_sb[:], bo[:, :])

            # ---- persistent activations ----
            # Q^T/K^T live per head on partitions 64-127 (base-64 K=64
            # matmuls sustain full rate; base-0 ones run at half rate).
            qt_h = [pers.tile([P, S], cdt, name=f"qth{h}")
                    for h in range(H_CORE)]
            kt_h = [pers.tile([P, S], cdt, name=f"kth{h}")
                    for h in range(H_CORE)]
            v_c = [pers.tile([P, JT, HPC, DK + 1], cdt, name=f"v{c}") for c in range(HC)]
            on_c = [pers.tile([P, S], cdt, name=f"on{c}") for c in range(HC)]

            for c in range(HC):
                ones_ap = v_c[c][:, :, :, DK:DK + 1]
                if dtype == "f32r":
                    ones_ap = ones_ap.bitcast(f32)
                nc.vector.memset(ones_ap, 1.0)

            # ---- projections ----
            def qk_proj(src, w_sb, b_sb, dst, chunks):
                for si in range(NSC):
                    ins = []
                    for kt in range(KT):
                        t = stream.tile([P, SC], cdt, tag="instream", bufs=12,
                                        name=f"in_{src.name}_{si}_{kt}_{chunks[0]}")
                        nc.sync.dma_start(
                            t[:], src[kt * P:(kt + 1) * P,
                                      si * SC:(si + 1) * SC])
                        ins.append(t)
                        yield
                    for c in chunks:
                        ps = psum.tile([P, SC], f32, tag="mm", bufs=4,
                                       name=f"ps_{src.name}_{si}_{c}")
                        for kt in range(KT):
                            nc.tensor.matmul(
                                ps[:],
                                lhsT=mm(w_sb[:, kt, c * P:(c + 1) * P]),
                                rhs=mm(ins[kt][:]),
                                start=(kt == 0), stop=(kt == KT - 1))
                            yield
                        stg = stream.tile([P, SC], cdt, tag="pstage", bufs=3,
                                          name=f"stg_{src.name}_{si}_{c}")
                        nc.vector.tensor_add(
                            stg[:], ps[:],
                            b_sb[:, c:c + 1].to_broadcast((P, SC)))
                        ssl = slice(si * SC, (si + 1) * SC)
                        nc.sync.dma_start(dst[c * HPC][DK:P, ssl],
                                          stg[0:DK, :])
                        nc.sync.dma_start(dst[c * HPC + 1][DK:P, ssl],
                                          stg[DK:P, :])
                        yield

            for g in (qk_proj(qT, wq_sb, bq_sb, qt_h, tuple(range(HC))),
                      qk_proj(kTd, wk_sb, bk_sb, kt_h, tuple(range(HC)))):
                for _ in g:
                    pass
            deferred = iter(())

            # V natural: psum[s, dh] = sum_k vT[k, s] * Wv[k, dh]
            for si in range(NSC):
                ins = []
                for kt in range(KT):
                    t = stream.tile([P, SC], cdt, tag="instream", bufs=12,
                                    name=f"in_v_{si}_{kt}")
                    nc.sync.dma_start(
                        t[:], vT[kt * P:(kt + 1) * P, si * SC:(si + 1) * SC])
                    ins.append(t)
                for sub in range(SC // P):
                    jt_idx = si * (SC // P) + sub
                    ps = psum.tile([P, DH], f32, tag="mm", bufs=4,
                                   name=f"ps_v_{jt_idx}")
                    for kt in range(KT):
                        nc.tensor.matmul(
                            ps[:],
                            lhsT=mm(ins[kt][:, sub * P:(sub + 1) * P]),
                            rhs=mm(wv_sb[:, kt, :]),
                            start=(kt == 0), stop=(kt == KT - 1))
                    for c in range(HC):
                        nc.vector.tensor_add(
                            v_c[c][:, jt_idx, :, 0:DK],
                            ps[:, c * P:(c + 1) * P].rearrange(
                                "p (h d) -> p h d", d=DK),
                            bvb_sb[:, c * HPC:(c + 1) * HPC, :])

            # ---- attention (flash over j, scores transposed) ----
            # Per-head blocks; sc has two buffers so scores(jt+1) overlap
            # exp(jt). AV matmuls trail one j-step so the PE program never
            # blocks the ACT engine behind unready work.
            for h in range(H_CORE):
                hc = h // HPC
                hh = h % HPC
                p0 = hh * DK
                for ibx in range(NIB):
                    i0 = ibx * ib
                    avs = [
                        psum.tile([P, SC], f32, tag="mm", bufs=4,
                                  name=f"av_{h}_{ibx}_{ic}")
                        for ic in range(ICB)
                    ]
                    e_ts = {}
                    for jt in range(JT + 1):
                        if jt < JT:
                            sc_t = psum.tile([P, ib], f32, tag="sc",
                                             bufs=2,
                                             name=f"sc_{h}_{ibx}_{jt}")
                            for ic in range(ICB):
                                nc.tensor.matmul(
                                    sc_t[:, ic * SC:(ic + 1) * SC],
                                    lhsT=mm(kt_h[h][DK:P,
                                                    jt * P:(jt + 1) * P]),
                                    rhs=mm(qt_h[h][DK:P,
                                                   i0 + ic * SC:i0 + (ic + 1) * SC]),
                                    start=True, stop=True)
                            e_t = stream.tile([P, ib], cdt, tag="e", bufs=3,
                                              name=f"e_{h}_{ibx}_{jt}")
                            nc.scalar.activation(e_t[:], sc_t[:], Exp,
                                                 bias=0.0, scale=scale)
                            e_ts[jt] = e_t
                        if jt >= 1:
                            pj = jt - 1
                            e_t = e_ts.pop(pj)
                            for ic in range(ICB):
                                nc.tensor.matmul(
                                    avs[ic][0:DK + 1, :],
                                    lhsT=mm(v_c[hc][:, pj, hh, :]),
                                    rhs=mm(e_t[:, ic * SC:(ic + 1) * SC]),
                                    start=(pj == 0), stop=(pj == JT - 1))
                    # drain AV psums to SBUF, normalize in the background
                    for ic in range(ICB):
                        av = avs[ic]
                        av_sb = stream.tile([P, SC], f32, tag="avsb", bufs=4,
                                            name=f"avsb_{h}_{ibx}_{ic}")
                        nc.vector.tensor_copy(av_sb[0:DK + 1, :],
                                              av[0:DK + 1, :])
                        rz = stream.tile([P, SC], f32, tag="rz", bufs=2,
                                         name=f"rz_{h}_{ibx}_{ic}")
                        nc.vector.reciprocal(rz[DK:DK + 1, :],
                                             av_sb[DK:DK + 1, :])
                        rz_d = dscratch.tile([1, SC], f32, tag="rzd", bufs=2,
                                             name=f"rzd_{h}_{ibx}_{ic}")
                        nc.sync.dma_start(rz_d[:], rz[DK:DK + 1, :])
                        rzb = stream.tile([P, SC], f32, tag="rzb", bufs=2,
                                          name=f"rzb_{h}_{ibx}_{ic}")
                        nc.sync.dma_start(
                            rzb[0:DK, :],
                            rz_d[:, :].to_broadcast((DK, SC)))
                        ot = stream.tile([P, SC], cdt, tag="ot", bufs=2,
                                         name=f"ot_{h}_{ibx}_{ic}")
                        nc.vector.tensor_mul(ot[0:DK, :], av_sb[0:DK, :],
                                             rzb[0:DK, :])
                        nc.sync.dma_start(
                            on_c[hc][p0:p0 + DK,
                                     i0 + ic * SC:i0 + (ic + 1) * SC],
                            ot[0:DK, :])

            # ---- output projection ----
            Ident = mybir.ActivationFunctionType.Identity
            for n in range(NOUT):
                for i in range(NSC):
                    idx = n * NSC + i
                    ps = psum.tile([P, SC], f32, tag=("sc", "mm")[idx % 2],
                                   bufs=(2, 4)[idx % 2],
                                   name=f"ps_o_{n}_{i}")
                    for c in range(HC):
                        nc.tensor.matmul(
                            ps[:],
                            lhsT=mm(wo_sb[:, c, n * P:(n + 1) * P]),
                            rhs=mm(on_c[c][:, i * SC:(i + 1) * SC]),
                            start=(c == 0), stop=(c == HC - 1))
                    o_sb = stream.tile([P, SC], f32, tag="osb", bufs=4,
                                       name=f"o_sb_{n}_{i}")
                    if idx % 2 == 0:
                        nc.scalar.activation(o_sb[:], ps[:], Ident,
                                             bias=bo_sb[:, n:n + 1],
                                             scale=1.0)
                    else:
                        nc.vector.tensor_add(
                            o_sb[:], ps[:],
                            bo_sb[:, n:n + 1].to_broadcast((P, SC)))
                    nc.sync.dma_start(
                        outT[n * P:(n + 1) * P, i * SC:(i + 1) * SC], o_sb[:])

    nc.finalize()
    return nc


def qT_ap_rearr(w_dram, p):
    """[D, N] dram weight -> [P, D//P, N] AP for SBUF load."""
    return w_dram[:, :].rearrange("(ko p) n -> p ko n", p=p)


def make_in_maps(query, key, value, Wq, bq, Wk, bk, Wv, bv, Wo, bo,
                 dtype="f32r"):
    """Shard full inputs into the 8 per-core input dicts."""
    f = lambda a: np.ascontiguousarray(np.asarray(a, dtype=np.float32))
    HC = DH // P
    NOUT = D // P
    query, key, value = f(query), f(key), f(value)
    Wq, Wk, Wv, Wo = f(Wq), f(Wk), f(Wv), f(Wo)
    bq, bk, bv, bo = f(bq), f(bk), f(bv), f(bo)
    if dtype == "bf16":
        import ml_dtypes
        cvt = lambda a: np.ascontiguousarray(a.astype(ml_dtypes.bfloat16))
    else:
        cvt = np.ascontiguousarray
    in_maps = []
    for core in range(N_CORES):
        b, g = core // GROUPS, core % GROUPS
        sl = slice(g * DH, (g + 1) * DH)
        in_maps.append({
            "qT": cvt(query[b].T),
            "kTd": cvt(key[b].T),
            "vT": cvt(value[b].T),
            "wq": cvt(Wq[:, sl]),
            "wk": cvt(Wk[:, sl]),
            "wv": cvt(Wv[:, sl]),
            "wo": cvt(Wo[sl, :]),
            "bq": np.ascontiguousarray(bq[sl].reshape(HC, P).T),
            "bk": np.ascontiguousarray(bk[sl].reshape(HC, P).T),
            "bvb": np.ascontiguousarray(
                np.broadcast_to(bv[sl].reshape(H_CORE, DK)[None], (P, H_CORE, DK))),
            "bo": (np.ascontiguousarray(bo.reshape(NOUT, P).T)
                   if g == 0 else np.zeros((P, NOUT), np.float32)),
        })
    return in_maps


# test hooks (ignored by the harness)
TRACE = False
LAST_RESULT = None
DTYPE = "bf16"
_NC_CACHE = {}


def kernel(query, key, value, Wq, bq, Wk, bk, Wv, bv, Wo, bo):
    global LAST_RESULT
    from concourse.bass_utils import run_bass_kernel_spmd

    if DTYPE not in _NC_CACHE:
        _NC_CACHE[DTYPE] = build_nc(dtype=DTYPE)
    nc = _NC_CACHE[DTYPE]

    in_maps = make_in_maps(query, key, value, Wq, bq, Wk, bk, Wv, bv, Wo, bo,
                           dtype=DTYPE)
    kwargs = {}
    if TRACE:
        kwargs = dict(trace=True, trace_cores=[0])
    res = run_bass_kernel_spmd(nc, in_maps, core_ids=list(range(N_CORES)), **kwargs)
    LAST_RESULT = res

    out = np.zeros((B, S, D), np.float32)
    for core in range(N_CORES):
        b = core // GROUPS
        out[b] += res.results[core]["outT"].T
    return out



# revision 9
# speedup vs baseline: 1.3037x; 1.3037x over previous
"""Multi-head attention, tensor-parallel across 8 Trainium2 NeuronCores.

Sharding: core = (batch b, head-group g), g covering 4 heads (256 dh).
Within a core heads are processed as PAIRS using 64x128 PE row-tiling:
head A of a pair lives on SBUF partitions 0-63, head B on 64-127, so the
two K=64 scores matmuls run concurrently on array tiles T0/T8, and each
AV matmul's K=128 contraction is split into top/bot halves on T0/T8
accumulating into one PSUM bank via has_written.

exp is split between ScalarE (exact, even j-tiles) and VectorE (odd
j-tiles) using a round-to-nearest int16 Schraudolph: bf16bits(exp(x)) ~
round(x*A + B), verified exact-convert on HW; its mean ratio error is
calibrated out (softmax cancels any residual common-mode bias).

Z comes from a ones column appended to V (AV psum row 64). Normalize:
reciprocal_approx_fast on Z, DRAM-roundtrip broadcast, DVE mults; head
B's normalized block is staged and DMA'd to partitions 64-127 so the
output projection keeps K=128.

Host: shards inputs, sums the 4 head-group partials per batch, adds bo.
"""

import os
import numpy as np

DBG_AV_SINGLE = os.environ.get("DBG_AV_SINGLE", "0") == "1"
DBG_RECIP_PLAIN = os.environ.get("DBG_RECIP_PLAIN", "0") == "1"
DBG_EXP_ACT = os.environ.get("DBG_EXP_ACT", "0") == "1"

B, S, D, H = 2, 2048, 1024, 16
DK = D // H              # 64 head dim
N_CORES = 8
GROUPS = N_CORES // B    # 4 head-groups
DH = D // GROUPS         # 256 head-dims per core (4 heads)
H_CORE = DH // DK        # 4 heads per core
SCALE = 1.0 / float(np.sqrt(DK))

P = 128                  # SBUF/PSUM partitions
SC = 512                 # matmul moving-dim chunk
IB = 512                 # flash i-block
LOG2E = float(np.log2(np.e))
SCH_A = float(128.0 * SCALE * LOG2E)       # schraudolph slope
SCH_B = float(127.0 * 128.0 - 7.35)        # schraudolph bias (mean-one)


def build_nc(S=S, D=D, DH=DH, DK=DK, scale=SCALE, ib=IB):
    import concourse.bacc as bacc
    import concourse.mybir as mybir
    import concourse.tile as tile

    f32 = mybir.dt.float32
    bf16 = mybir.dt.bfloat16
    i16 = mybir.dt.int16
    Exp = mybir.ActivationFunctionType.Exp
    Ident = mybir.ActivationFunctionType.Identity
    Mult = mybir.AluOpType.mult
    Add = mybir.AluOpType.add
    cdt = bf16

    KT = D // P                    # contraction tiles for projections (8)
    NSC = S // SC                  # s chunks (4)
    HC = DH // P                   # head pairs (2)
    HPC = P // DK                  # heads per pair (2)
    JT = S // P                    # j tiles (16)
    NIB = S // ib                  # i blocks (4)
    NOUT = D // P                  # output row chunks (8)
    LAG = 2                        # AV trails scores by LAG j-steps

    nc = bacc.Bacc("TRN2", target_bir_lowering=False, debug=False)

    qT = nc.dram_tensor("qT", [D, S], cdt, kind="ExternalInput")
    kTd = nc.dram_tensor("kTd", [D, S], cdt, kind="ExternalInput")
    vT = nc.dram_tensor("vT", [D, S], cdt, kind="ExternalInput")
    wq = nc.dram_tensor("wq", [D, DH], cdt, kind="ExternalInput")
    wk = nc.dram_tensor("wk", [D, DH], cdt, kind="ExternalInput")
    wv = nc.dram_tensor("wv", [D, DH], cdt, kind="ExternalInput")
    wo = nc.dram_tensor("wo", [DH, D], cdt, kind="ExternalInput")
    bq = nc.dram_tensor("bq", [P, HC], f32, kind="ExternalInput")
    bk = nc.dram_tensor("bk", [P, HC], f32, kind="ExternalInput")
    bvb = nc.dram_tensor("bvb", [P, H_CORE, DK], f32, kind="ExternalInput")
    outT = nc.dram_tensor("outT", [D, S], cdt, kind="ExternalOutput")

    with tile.TileContext(nc) as tc:
        with (
            tc.tile_pool(name="const", bufs=1) as cpool,
            tc.tile_pool(name="pers", bufs=1) as pers,
            tc.tile_pool(name="stream", bufs=1) as stream,
            tc.tile_pool(name="psum", bufs=1, space="PSUM") as psum,
            tc.tile_pool(name="dscratch", bufs=1, space="DRAM") as dscratch,
        ):
            # ---- constants ----
            wq_sb = cpool.tile([P, KT, DH], cdt, name="wq_sb")
            wk_sb = cpool.tile([P, KT, DH], cdt, name="wk_sb")
            wv_sb = cpool.tile([P, KT, DH], cdt, name="wv_sb")
            wo_sb = cpool.tile([P, HC, D], cdt, name="wo_sb")
            bq_sb = cpool.tile([P, HC], f32, name="bq_sb")
            bk_sb = cpool.tile([P, HC], f32, name="bk_sb")
            bvb_sb = cpool.tile([P, H_CORE, DK], f32, name="bvb_sb")
            nc.sync.dma_start(wq_sb[:], wq[:, :].rearrange("(ko p) n -> p ko n", p=P))
            nc.sync.dma_start(wk_sb[:], wk[:, :].rearrange("(ko p) n -> p ko n", p=P))
            nc.sync.dma_start(wv_sb[:], wv[:, :].rearrange("(ko p) n -> p ko n", p=P))
            nc.sync.dma_start(wo_sb[:], wo[:, :].rearrange("(c p) n -> p c n", p=P))
            nc.sync.dma_start(bq_sb[:], bq[:, :])
            nc.sync.dma_start(bk_sb[:], bk[:, :])
            nc.sync.dma_start(bvb_sb[:], bvb[:, :, :])

            # ---- persistent activations (head-pair layout) ----
            # qt/kt pair c: rows 0-63 = head 2c (dk dims), rows 64-127 =
            # head 2c+1. v pair c: rows = j within tile, + ones column.
            qt = [pers.tile([P, S], cdt, name=f"qt{c}") for c in range(HC)]
            kt = [pers.tile([P, S], cdt, name=f"kt{c}") for c in range(HC)]
            v_c = [pers.tile([P, JT, HPC, DK + 1], cdt, name=f"v{c}")
                   for c in range(HC)]
            on_c = [pers.tile([P, S], cdt, name=f"on{c}") for c in range(HC)]

            for c in range(HC):
                nc.vector.memset(v_c[c][:, :, :, DK:DK + 1], 1.0)

            # ---- Q/K projections ----
            def qk_proj(src, w_sb, b_sb, dst):
                for si in range(NSC):
                    ins = []
                    for kti in range(KT):
                        t = stream.tile([P, SC], cdt, tag="instream", bufs=12,
                                        name=f"in_{src.name}_{si}_{kti}")
                        nc.sync.dma_start(
                            t[:], src[kti * P:(kti + 1) * P,
                                      si * SC:(si + 1) * SC])
                        ins.append(t)
                    ps = psum.tile([P, 2 * SC], f32, tag="sc", bufs=2,
                                   name=f"ps_{src.name}_{si}")
                    for c in range(HC):
                        for kti in range(KT):
                            nc.tensor.matmul(
                                ps[:, c * SC:(c + 1) * SC],
                                lhsT=w_sb[:, kti, c * P:(c + 1) * P],
                                rhs=ins[kti][:],
                                start=(kti == 0), stop=(kti == KT - 1))
                    ssl = slice(si * SC, (si + 1) * SC)
                    # evac + bias: head-pair chunk c goes straight to dst[c]
                    nc.vector.tensor_add(
                        dst[0][:, ssl], ps[:, 0:SC],
                        b_sb[:, 0:1].to_broadcast((P, SC)))
                    nc.scalar.activation(
                        dst[1][:, ssl], ps[:, SC:2 * SC], Ident,
                        bias=b_sb[:, 1:2], scale=1.0)

            qk_proj(qT, wq_sb, bq_sb, qt)
            qk_proj(kTd, wk_sb, bk_sb, kt)

            # ---- V projection (natural [j, dh]) ----
            for si in range(NSC):
                ins = []
                for kti in range(KT):
                    t = stream.tile([P, SC], cdt, tag="instream", bufs=12,
                                    name=f"in_v_{si}_{kti}")
                    nc.sync.dma_start(
                        t[:], vT[kti * P:(kti + 1) * P, si * SC:(si + 1) * SC])
                    ins.append(t)
                for sub in range(SC // P):
                    jt_idx = si * (SC // P) + sub
                    ps = psum.tile([P, 2 * SC], f32, tag="sc", bufs=2,
                                   name=f"ps_v_{jt_idx}")
                    for kti in range(KT):
                        nc.tensor.matmul(
                            ps[:, 0:DH],
                            lhsT=ins[kti][:, sub * P:(sub + 1) * P],
                            rhs=wv_sb[:, kti, :],
                            start=(kti == 0), stop=(kti == KT - 1))
                    for c in range(HC):
                        src_ap = ps[:, c * P:(c + 1) * P].rearrange(
                            "p (h d) -> p h d", d=DK)
                        dst_ap = v_c[c][:, jt_idx, :, 0:DK]
                        bias_ap = bvb_sb[:, c * HPC:(c + 1) * HPC, :]
                        nc.vector.tensor_add(dst_ap, src_ap, bias_ap)

            # ---- attention (flash over j; head pairs on T0/T8) ----
            for c in range(HC):
                for ibx in range(NIB):
                    i0 = ibx * ib
                    isl = slice(i0, i0 + ib)
                    av = psum.tile([P, 2 * SC], f32, tag="av", bufs=2,
                                   name=f"av_{c}_{ibx}")
                    e_ts = {}
                    for jt in range(JT + LAG):
                        if jt < JT:
                            sct = psum.tile([P, 2 * SC], f32, tag="sc",
                                            bufs=2, name=f"sc_{c}_{ibx}_{jt}")
                            jsl = slice(jt * P, (jt + 1) * P)
                            nc.tensor.matmul(
                                sct[:, 0:SC],
                                lhsT=kt[c][0:DK, jsl],
                                rhs=qt[c][0:DK, isl],
                                start=True, stop=True)
                            nc.tensor.matmul(
                                sct[:, SC:2 * SC],
                                lhsT=kt[c][DK:P, jsl],
                                rhs=qt[c][DK:P, isl],
                                start=True, stop=True)
                            et = stream.tile([P, 2 * SC], cdt, tag="e",
                                             bufs=4, name=f"e_{c}_{ibx}_{jt}")
                            if jt % 2 == 0 or DBG_EXP_ACT:
                                nc.scalar.activation(et[:], sct[:], Exp,
                                                     bias=0.0, scale=scale)
                            else:
                                nc.vector.tensor_scalar(
                                    et[:].bitcast(i16), sct[:],
                                    SCH_A, SCH_B, Mult, Add)
                            e_ts[jt] = et
                        if jt >= LAG:
                            pj = jt - LAG
                            et = e_ts.pop(pj)
                            st, sp = (pj == 0), (pj == JT - 1)
                            if DBG_AV_SINGLE:
                                for h in range(HPC):
                                    nc.tensor.matmul(
                                        av[0:DK + 1, h * SC:(h + 1) * SC],
                                        lhsT=v_c[c][:, pj, h, :],
                                        rhs=et[:, h * SC:(h + 1) * SC],
                                        start=st, stop=sp)
                            else:
                                # A-top(T0), B-bot(T8), B-top(T0), A-bot(T8)
                                nc.tensor.matmul(
                                    av[0:DK + 1, 0:SC],
                                    lhsT=v_c[c][0:DK, pj, 0, :],
                                    rhs=et[0:DK, 0:SC],
                                    start=st, stop=False)
                                nc.tensor.matmul(
                                    av[0:DK + 1, SC:2 * SC],
                                    lhsT=v_c[c][DK:P, pj, 1, :],
                                    rhs=et[DK:P, SC:2 * SC],
                                    start=st, stop=False)
                                nc.tensor.matmul(
                                    av[0:DK + 1, SC:2 * SC],
                                    lhsT=v_c[c][0:DK, pj, 1, :],
                                    rhs=et[0:DK, SC:2 * SC],
                                    start=False, stop=sp)
                                nc.tensor.matmul(
                                    av[0:DK + 1, 0:SC],
                                    lhsT=v_c[c][DK:P, pj, 0, :],
                                    rhs=et[DK:P, 0:SC],
                                    start=False, stop=sp)
                    # ---- normalize (trails into next block) ----
                    # Z row -> SBUF (ACT), DRAM-reshape to [128, 8] for a
                    # cheap all-lane reciprocal, then broadcast-load.
                    zrow = stream.tile([P, 2 * SC], f32, tag="rz", bufs=2,
                                       name=f"rz_{c}_{ibx}")
                    nc.scalar.copy(zrow[DK:DK + 1, :], av[DK:DK + 1, :])
                    z_d = dscratch.tile([1, 2 * SC], f32, tag="zd", bufs=2,
                                        name=f"zd_{c}_{ibx}")
                    nc.sync.dma_start(z_d[:], zrow[DK:DK + 1, :])
                    zc = stream.tile([P, 2 * (2 * SC) // P], f32, tag="zc",
                                     bufs=2, name=f"zc_{c}_{ibx}")
                    zw = (2 * SC) // P
                    nc.sync.dma_start(
                        zc[:, 0:zw],
                        z_d[:, :].rearrange("o (p x) -> (o p) x", p=P))
                    nc.vector.reciprocal(zc[:, zw:2 * zw], zc[:, 0:zw])
                    rz_d = dscratch.tile([1, 2 * SC], f32, tag="rzd", bufs=2,
                                         name=f"rzd_{c}_{ibx}")
                    nc.sync.dma_start(
                        rz_d[:, :].rearrange("o (p x) -> (o p) x", p=P),
                        zc[:, zw:2 * zw])
                    rzb = stream.tile([DK, 2 * SC], f32, tag="rzb", bufs=2,
                                      name=f"rzb_{c}_{ibx}")
                    nc.sync.dma_start(
                        rzb[0:DK, :], rz_d[:, :].to_broadcast((DK, 2 * SC)))
                    nc.vector.tensor_mul(on_c[c][0:DK, isl],
                                         av[0:DK, 0:SC], rzb[0:DK, 0:SC])
                    stg = stream.tile([DK, SC], cdt, tag="stgB", bufs=2,
                                      name=f"stg_{c}_{ibx}")
                    nc.vector.tensor_mul(stg[0:DK, :],
                                         av[0:DK, SC:2 * SC],
                                         rzb[0:DK, SC:2 * SC])
                    nc.sync.dma_start(on_c[c][DK:P, isl], stg[0:DK, :])

            # ---- output projection (bias added on host) ----
            for i in range(NSC):
                for n in range(NOUT):
                    idx = i * NOUT + n
                    pso = psum.tile([P, 2 * SC], f32,
                                    tag=("sc", "av")[idx % 2], bufs=2,
                                    name=f"ps_o_{n}_{i}")
                    for c in range(HC):
                        nc.tensor.matmul(
                            pso[:, 0:SC],
                            lhsT=wo_sb[:, c, n * P:(n + 1) * P],
                            rhs=on_c[c][:, i * SC:(i + 1) * SC],
                            start=(c == 0), stop=(c == HC - 1))
                    o_sb = stream.tile([P, SC], cdt, tag="osb", bufs=4,
                                       name=f"o_sb_{n}_{i}")
                    if idx % 2 == 0:
                        nc.scalar.copy(o_sb[:], pso[:, 0:SC])
                    else:
                        nc.vector.tensor_copy(o_sb[:], pso[:, 0:SC])
                    nc.sync.dma_start(
                        outT[n * P:(n + 1) * P, i * SC:(i + 1) * SC], o_sb[:])

    nc.finalize()
    return nc


def make_in_maps(query, key, value, Wq, bq, Wk, bk, Wv, bv, Wo, bo):
    """Shard full inputs into the 8 per-core input dicts."""
    import ml_dtypes
    f = lambda a: np.ascontiguousarray(np.asarray(a, dtype=np.float32))
    HC = DH // P
    query, key, value = f(query), f(key), f(value)
    Wq, Wk, Wv, Wo = f(Wq), f(Wk), f(Wv), f(Wo)
    bq, bk, bv = f(bq), f(bk), f(bv)
    cvt = lambda a: np.ascontiguousarray(a.astype(ml_dtypes.bfloat16))
    in_maps = []
    for core in range(N_CORES):
        b, g = core // GROUPS, core % GROUPS
        sl = slice(g * DH, (g + 1) * DH)
        in_maps.append({
            "qT": cvt(query[b].T),
            "kTd": cvt(key[b].T),
            "vT": cvt(value[b].T),
            "wq": cvt(Wq[:, sl]),
            "wk": cvt(Wk[:, sl]),
            "wv": cvt(Wv[:, sl]),
            "wo": cvt(Wo[sl, :]),
            "bq": np.ascontiguousarray(bq[sl].reshape(HC, P).T),
            "bk": np.ascontiguousarray(bk[sl].reshape(HC, P).T),
            "bvb": np.ascontiguousarray(
                np.broadcast_to(bv[sl].reshape(H_CORE, DK)[None],
                                (P, H_CORE, DK))),
        })
    return in_maps


# test hooks (ignored by the harness)
TRACE = False
LAST_RESULT = None
DTYPE = "bf16"
_NC_CACHE = {}


def kernel(query, key, value, Wq, bq, Wk, bk, Wv, bv, Wo, bo):
    global LAST_RESULT
    from concourse.bass_utils import run_bass_kernel_spmd

    if "nc" not in _NC_CACHE:
        _NC_CACHE["nc"] = build_nc()
    nc = _NC_CACHE["nc"]

    in_maps = make_in_maps(query, key, value, Wq, bq, Wk, bk, Wv, bv, Wo, bo)
    kwargs = {}
    if TRACE:
        kwargs = dict(trace=True, trace_cores=[0])
    res = run_bass_kernel_spmd(nc, in_maps, core_ids=list(range(N_CORES)),
                               **kwargs)
    LAST_RESULT = res

    out = np.zeros((B, S, D), np.float32)
    for core in range(N_CORES):
        b = core // GROUPS
        out[b] += res.results[core]["outT"].T.astype(np.float32)
    out += np.asarray(bo, dtype=np.float32)
    return out


# revision 11
# speedup vs baseline: 1.3709x; 1.0516x over previous
"""Multi-head attention, tensor-parallel across 8 Trainium2 NeuronCores.

Sharding: core = (batch b, head-group g), g covering 4 heads (256 dh).
Within a core heads are processed as PAIRS using 64x128 PE row-tiling:
head A of a pair lives on SBUF partitions 0-63, head B on 64-127, so the
two K=64 scores matmuls run concurrently on array tiles T0/T8, and each
AV matmul's K=128 contraction is split into top/bot halves on T0/T8
accumulating into one PSUM bank via has_written.

exp is split between ScalarE (exact, even j-tiles) and VectorE (odd
j-tiles) using a round-to-nearest int16 Schraudolph: bf16bits(exp(x)) ~
round(x*A + B), verified exact-convert on HW; its mean ratio error is
calibrated out (softmax cancels any residual common-mode bias).

Z comes from a ones column appended to V (AV psum row 64). Normalize:
reciprocal_approx_fast on Z, DRAM-roundtrip broadcast, DVE mults; head
B's normalized block is staged and DMA'd to partitions 64-127 so the
output projection keeps K=128.

Host: shards inputs, sums the 4 head-group partials per batch, adds bo.
"""

import os
import numpy as np

DBG_AV_SINGLE = os.environ.get("DBG_AV_SINGLE", "0") == "1"
DBG_RECIP_PLAIN = os.environ.get("DBG_RECIP_PLAIN", "0") == "1"
DBG_EXP_ACT = os.environ.get("DBG_EXP_ACT", "0") == "1"

B, S, D, H = 2, 2048, 1024, 16
DK = D // H              # 64 head dim
N_CORES = 8
GROUPS = N_CORES // B    # 4 head-groups
DH = D // GROUPS         # 256 head-dims per core (4 heads)
H_CORE = DH // DK        # 4 heads per core
SCALE = 1.0 / float(np.sqrt(DK))

P = 128                  # SBUF/PSUM partitions
SC = 512                 # matmul moving-dim chunk
IB = 512                 # flash i-block
LOG2E = float(np.log2(np.e))
SCH_A = float(128.0 * SCALE * LOG2E)       # schraudolph slope
SCH_B = float(127.0 * 128.0 - 7.35)        # schraudolph bias (mean-one)


def build_nc(S=S, D=D, DH=DH, DK=DK, scale=SCALE, ib=IB):
    import concourse.bacc as bacc
    import concourse.mybir as mybir
    import concourse.tile as tile

    f32 = mybir.dt.float32
    bf16 = mybir.dt.bfloat16
    i16 = mybir.dt.int16
    Exp = mybir.ActivationFunctionType.Exp
    Ident = mybir.ActivationFunctionType.Identity
    Mult = mybir.AluOpType.mult
    Add = mybir.AluOpType.add
    cdt = bf16

    KT = D // P                    # contraction tiles for projections (8)
    NSC = S // SC                  # s chunks (4)
    HC = DH // P                   # head pairs (2)
    HPC = P // DK                  # heads per pair (2)
    JT = S // P                    # j tiles (16)
    NIB = S // ib                  # i blocks (4)
    NOUT = D // P                  # output row chunks (8)
    LAG = 2                        # AV trails scores by LAG j-steps

    nc = bacc.Bacc("TRN2", target_bir_lowering=False, debug=False)

    qT = nc.dram_tensor("qT", [D, S], cdt, kind="ExternalInput")
    kTd = nc.dram_tensor("kTd", [D, S], cdt, kind="ExternalInput")
    vT = nc.dram_tensor("vT", [D, S], cdt, kind="ExternalInput")
    wq = nc.dram_tensor("wq", [D, DH], cdt, kind="ExternalInput")
    wk = nc.dram_tensor("wk", [D, DH], cdt, kind="ExternalInput")
    wv = nc.dram_tensor("wv", [D, DH], cdt, kind="ExternalInput")
    wo = nc.dram_tensor("wo", [DH, D], cdt, kind="ExternalInput")
    bq = nc.dram_tensor("bq", [P, HC], f32, kind="ExternalInput")
    bk = nc.dram_tensor("bk", [P, HC], f32, kind="ExternalInput")
    bvb = nc.dram_tensor("bvb", [P, H_CORE, DK], f32, kind="ExternalInput")
    outT = nc.dram_tensor("outT", [D, S], cdt, kind="ExternalOutput")

    with tile.TileContext(nc) as tc:
        with (
            tc.tile_pool(name="const", bufs=1) as cpool,
            tc.tile_pool(name="pers", bufs=1) as pers,
            tc.tile_pool(name="stream", bufs=1) as stream,
            tc.tile_pool(name="psum", bufs=1, space="PSUM") as psum,
            tc.tile_pool(name="dscratch", bufs=1, space="DRAM") as dscratch,
        ):
            # ---- constants ----
            wq_sb = cpool.tile([P, KT, DH], cdt, name="wq_sb")
            wk_sb = cpool.tile([P, KT, DH], cdt, name="wk_sb")
            wv_sb = cpool.tile([P, KT, DH], cdt, name="wv_sb")
            wo_sb = cpool.tile([P, HC, D], cdt, name="wo_sb")
            bq_sb = cpool.tile([P, HC], f32, name="bq_sb")
            bk_sb = cpool.tile([P, HC], f32, name="bk_sb")
            bvb_sb = cpool.tile([P, H_CORE, DK], f32, name="bvb_sb")
            nc.sync.dma_start(wq_sb[:], wq[:, :].rearrange("(ko p) n -> p ko n", p=P))
            nc.sync.dma_start(wk_sb[:], wk[:, :].rearrange("(ko p) n -> p ko n", p=P))
            nc.sync.dma_start(wv_sb[:], wv[:, :].rearrange("(ko p) n -> p ko n", p=P))
            nc.sync.dma_start(wo_sb[:], wo[:, :].rearrange("(c p) n -> p c n", p=P))
            nc.sync.dma_start(bq_sb[:], bq[:, :])
            nc.sync.dma_start(bk_sb[:], bk[:, :])
            nc.sync.dma_start(bvb_sb[:], bvb[:, :, :])

            # ---- persistent activations (head-pair layout) ----
            # qt/kt pair c: rows 0-63 = head 2c (dk dims), rows 64-127 =
            # head 2c+1. v pair c: rows = j within tile, + ones column.
            qt = [pers.tile([P, S], cdt, name=f"qt{c}") for c in range(HC)]
            kt = [pers.tile([P, S], cdt, name=f"kt{c}") for c in range(HC)]
            v_c = [pers.tile([P, JT, HPC, DK + 1], cdt, name=f"v{c}")
                   for c in range(HC)]
            on_c = [pers.tile([P, S], cdt, name=f"on{c}") for c in range(HC)]

            for c in range(HC):
                nc.vector.memset(v_c[c][:, :, :, DK:DK + 1], 1.0)

            # ---- projections (inputs loaded as full-row 512KB DMAs) ----
            def load_tensor(src):
                bt = stream.tile([P, KT, S], cdt, tag="big_in", bufs=2,
                                 name=f"bi_{src.name}")
                for kti in range(KT):
                    nc.sync.dma_start(bt[:, kti, :],
                                      src[kti * P:(kti + 1) * P, :])
                return bt

            def qk_proj(src, w_sb, b_sb, dst):
                bt = load_tensor(src)
                for si in range(NSC):
                    ps = psum.tile([P, 2 * SC], f32, tag="sc", bufs=2,
                                   name=f"ps_{src.name}_{si}")
                    ssl = slice(si * SC, (si + 1) * SC)
                    for c in range(HC):
                        for kti in range(KT):
                            nc.tensor.matmul(
                                ps[:, c * SC:(c + 1) * SC],
                                lhsT=w_sb[:, kti, c * P:(c + 1) * P],
                                rhs=bt[:, kti, ssl],
                                start=(kti == 0), stop=(kti == KT - 1))
                    # evac + bias: head-pair chunk c goes straight to dst[c]
                    nc.vector.tensor_add(
                        dst[0][:, ssl], ps[:, 0:SC],
                        b_sb[:, 0:1].to_broadcast((P, SC)))
                    nc.scalar.activation(
                        dst[1][:, ssl], ps[:, SC:2 * SC], Ident,
                        bias=b_sb[:, 1:2], scale=1.0)

            qk_proj(qT, wq_sb, bq_sb, qt)
            qk_proj(kTd, wk_sb, bk_sb, kt)

            # ---- V projection (natural [j, dh]) ----
            vin = load_tensor(vT)
            for si in range(NSC):
                for sub in range(SC // P):
                    jt_idx = si * (SC // P) + sub
                    ps = psum.tile([P, 2 * SC], f32, tag="sc", bufs=2,
                                   name=f"ps_v_{jt_idx}")
                    jsl = slice(si * SC + sub * P, si * SC + (sub + 1) * P)
                    for kti in range(KT):
                        nc.tensor.matmul(
                            ps[:, 0:DH],
                            lhsT=vin[:, kti, jsl],
                            rhs=wv_sb[:, kti, :],
                            start=(kti == 0), stop=(kti == KT - 1))
                    for c in range(HC):
                        src_ap = ps[:, c * P:(c + 1) * P].rearrange(
                            "p (h d) -> p h d", d=DK)
                        dst_ap = v_c[c][:, jt_idx, :, 0:DK]
                        bias_ap = bvb_sb[:, c * HPC:(c + 1) * HPC, :]
                        nc.vector.tensor_add(dst_ap, src_ap, bias_ap)

            # ---- attention (flash over j; head pairs on T0/T8) ----
            for c in range(HC):
                for ibx in range(NIB):
                    i0 = ibx * ib
                    isl = slice(i0, i0 + ib)
                    av = psum.tile([P, 2 * SC], f32, tag="av", bufs=2,
                                   name=f"av_{c}_{ibx}")
                    e_ts = {}
                    for jt in range(JT + LAG):
                        if jt < JT:
                            sct = psum.tile([P, 2 * SC], f32, tag="sc",
                                            bufs=2, name=f"sc_{c}_{ibx}_{jt}")
                            jsl = slice(jt * P, (jt + 1) * P)
                            nc.tensor.matmul(
                                sct[:, 0:SC],
                                lhsT=kt[c][0:DK, jsl],
                                rhs=qt[c][0:DK, isl],
                                start=True, stop=True)
                            nc.tensor.matmul(
                                sct[:, SC:2 * SC],
                                lhsT=kt[c][DK:P, jsl],
                                rhs=qt[c][DK:P, isl],
                                start=True, stop=True)
                            et = stream.tile([P, 2 * SC], cdt, tag="e",
                                             bufs=4, name=f"e_{c}_{ibx}_{jt}")
                            if jt % 2 == 0 or DBG_EXP_ACT:
                                nc.scalar.activation(et[:], sct[:], Exp,
                                                     bias=0.0, scale=scale)
                            else:
                                nc.vector.tensor_scalar(
                                    et[:].bitcast(i16), sct[:],
                                    SCH_A, SCH_B, Mult, Add)
                            e_ts[jt] = et
                        if jt >= LAG:
                            pj = jt - LAG
                            et = e_ts.pop(pj)
                            st, sp = (pj == 0), (pj == JT - 1)
                            if DBG_AV_SINGLE:
                                for h in range(HPC):
                                    nc.tensor.matmul(
                                        av[0:DK + 1, h * SC:(h + 1) * SC],
                                        lhsT=v_c[c][:, pj, h, :],
                                        rhs=et[:, h * SC:(h + 1) * SC],
                                        start=st, stop=sp)
                            else:
                                # A-top(T0), B-bot(T8), B-top(T0), A-bot(T8)
                                nc.tensor.matmul(
                                    av[0:DK + 1, 0:SC],
                                    lhsT=v_c[c][0:DK, pj, 0, :],
                                    rhs=et[0:DK, 0:SC],
                                    start=st, stop=False)
                                nc.tensor.matmul(
                                    av[0:DK + 1, SC:2 * SC],
                                    lhsT=v_c[c][DK:P, pj, 1, :],
                                    rhs=et[DK:P, SC:2 * SC],
                                    start=st, stop=False)
                                nc.tensor.matmul(
                                    av[0:DK + 1, SC:2 * SC],
                                    lhsT=v_c[c][0:DK, pj, 1, :],
                                    rhs=et[0:DK, SC:2 * SC],
                                    start=False, stop=sp)
                                nc.tensor.matmul(
                                    av[0:DK + 1, 0:SC],
                                    lhsT=v_c[c][DK:P, pj, 0, :],
                                    rhs=et[DK:P, 0:SC],
                                    start=False, stop=sp)
                    # ---- normalize (trails into next block) ----
                    # Z row -> SBUF (ACT), DRAM-reshape to [128, 8] for a
                    # cheap all-lane reciprocal, then broadcast-load.
                    zrow = stream.tile([P, 2 * SC], f32, tag="rz", bufs=2,
                                       name=f"rz_{c}_{ibx}")
                    nc.scalar.copy(zrow[DK:DK + 1, :], av[DK:DK + 1, :])
                    z_d = dscratch.tile([1, 2 * SC], f32, tag="zd", bufs=2,
                                        name=f"zd_{c}_{ibx}")
                    nc.sync.dma_start(z_d[:], zrow[DK:DK + 1, :])
                    zc = stream.tile([P, 2 * (2 * SC) // P], f32, tag="zc",
                                     bufs=2, name=f"zc_{c}_{ibx}")
                    zw = (2 * SC) // P
                    nc.sync.dma_start(
                        zc[:, 0:zw],
                        z_d[:, :].rearrange("o (p x) -> (o p) x", p=P))
                    nc.vector.reciprocal(zc[:, zw:2 * zw], zc[:, 0:zw])
                    rz_d = dscratch.tile([1, 2 * SC], f32, tag="rzd", bufs=2,
                                         name=f"rzd_{c}_{ibx}")
                    nc.sync.dma_start(
                        rz_d[:, :].rearrange("o (p x) -> (o p) x", p=P),
                        zc[:, zw:2 * zw])
                    rzb = stream.tile([DK, 2 * SC], f32, tag="rzb", bufs=2,
                                      name=f"rzb_{c}_{ibx}")
                    nc.sync.dma_start(
                        rzb[0:DK, :], rz_d[:, :].to_broadcast((DK, 2 * SC)))
                    nc.vector.tensor_mul(on_c[c][0:DK, isl],
                                         av[0:DK, 0:SC], rzb[0:DK, 0:SC])
                    stg = stream.tile([DK, SC], cdt, tag="stgB", bufs=2,
                                      name=f"stg_{c}_{ibx}")
                    nc.vector.tensor_mul(stg[0:DK, :],
                                         av[0:DK, SC:2 * SC],
                                         rzb[0:DK, SC:2 * SC])
                    nc.sync.dma_start(on_c[c][DK:P, isl], stg[0:DK, :])

            # ---- output projection (bias added on host) ----
            for n in range(NOUT):
                o_stg = stream.tile([P, S], cdt, tag="ostg", bufs=2,
                                    name=f"ostg_{n}")
                for i in range(NSC):
                    idx = n * NSC + i
                    pso = psum.tile([P, 2 * SC], f32,
                                    tag=("sc", "av")[idx % 2], bufs=2,
                                    name=f"ps_o_{n}_{i}")
                    for c in range(HC):
                        nc.tensor.matmul(
                            pso[:, 0:SC],
                            lhsT=wo_sb[:, c, n * P:(n + 1) * P],
                            rhs=on_c[c][:, i * SC:(i + 1) * SC],
                            start=(c == 0), stop=(c == HC - 1))
                    osl = slice(i * SC, (i + 1) * SC)
                    if idx % 2 == 0:
                        nc.scalar.copy(o_stg[:, osl], pso[:, 0:SC])
                    else:
                        nc.vector.tensor_copy(o_stg[:, osl], pso[:, 0:SC])
                nc.sync.dma_start(outT[n * P:(n + 1) * P, :], o_stg[:])

    nc.finalize()
    return nc


def make_in_maps(query, key, value, Wq, bq, Wk, bk, Wv, bv, Wo, bo):
    """Shard full inputs into the 8 per-core input dicts."""
    import ml_dtypes
    f = lambda a: np.ascontiguousarray(np.asarray(a, dtype=np.float32))
    HC = DH // P
    query, key, value = f(query), f(key), f(value)
    Wq, Wk, Wv, Wo = f(Wq), f(Wk), f(Wv), f(Wo)
    bq, bk, bv = f(bq), f(bk), f(bv)
    cvt = lambda a: np.ascontiguousarray(a.astype(ml_dtypes.bfloat16))
    in_maps = []
    for core in range(N_CORES):
        b, g = core // GROUPS, core % GROUPS
        sl = slice(g * DH, (g + 1) * DH)
        in_maps.append({
            "qT": cvt(query[b].T),
            "kTd": cvt(key[b].T),
            "vT": cvt(value[b].T),
            "wq": cvt(Wq[:, sl]),
            "wk": cvt(Wk[:, sl]),
            "wv": cvt(Wv[:, sl]),
            "wo": cvt(Wo[sl, :]),
            "bq": np.ascontiguousarray(bq[sl].reshape(HC, P).T),
            "bk": np.ascontiguousarray(bk[sl].reshape(HC, P).T),
            "bvb": np.ascontiguousarray(
                np.broadcast_to(bv[sl].reshape(H_CORE, DK)[None],
                                (P, H_CORE, DK))),
        })
    return in_maps


# test hooks (ignored by the harness)
TRACE = False
LAST_RESULT = None
DTYPE = "bf16"
_NC_CACHE = {}


def kernel(query, key, value, Wq, bq, Wk, bk, Wv, bv, Wo, bo):
    global LAST_RESULT
    from concourse.bass_utils import run_bass_kernel_spmd

    if "nc" not in _NC_CACHE:
        _NC_CACHE["nc"] = build_nc()
    nc = _NC_CACHE["nc"]

    in_maps = make_in_maps(query, key, value, Wq, bq, Wk, bk, Wv, bv, Wo, bo)
    kwargs = {}
    if TRACE:
        kwargs = dict(trace=True, trace_cores=[0])
    res = run_bass_kernel_spmd(nc, in_maps, core_ids=list(range(N_CORES)),
                               **kwargs)
    LAST_RESULT = res

    out = np.zeros((B, S, D), np.float32)
    for core in range(N_CORES):
        b = core // GROUPS
        out[b] += res.results[core]["outT"].T.astype(np.float32)
    out += np.asarray(bo, dtype=np.float32)
    return out


# revision 15
# speedup vs baseline: 1.4142x; 1.0316x over previous
"""Multi-head attention, tensor-parallel across 8 Trainium2 NeuronCores.

Sharding: core = (batch b, head-group g), g covering 4 heads (256 dh).
Within a core heads are processed as PAIRS using 64x128 PE row-tiling:
head A of a pair lives on SBUF partitions 0-63, head B on 64-127, so the
two K=64 scores matmuls run concurrently on array tiles T0/T8, and each
AV matmul's K=128 contraction is split into top/bot halves on T0/T8
accumulating into one PSUM bank via has_written.

exp is split between ScalarE (exact, even j-tiles) and VectorE (odd
j-tiles) using a round-to-nearest int16 Schraudolph: bf16bits(exp(x)) ~
round(x*A + B), verified exact-convert on HW; its mean ratio error is
calibrated out (softmax cancels any residual common-mode bias).

Z comes from a ones column appended to V (AV psum row 64). Normalize:
reciprocal_approx_fast on Z, DRAM-roundtrip broadcast, DVE mults; head
B's normalized block is staged and DMA'd to partitions 64-127 so the
output projection keeps K=128.

Host: shards inputs, sums the 4 head-group partials per batch, adds bo.
"""

import os
import numpy as np

DBG_AV_SINGLE = os.environ.get("DBG_AV_SINGLE", "0") == "1"
DBG_RECIP_PLAIN = os.environ.get("DBG_RECIP_PLAIN", "0") == "1"
DBG_EXP_ACT = os.environ.get("DBG_EXP_ACT", "0") == "1"

B, S, D, H = 2, 2048, 1024, 16
DK = D // H              # 64 head dim
N_CORES = 8
GROUPS = N_CORES // B    # 4 head-groups
DH = D // GROUPS         # 256 head-dims per core (4 heads)
H_CORE = DH // DK        # 4 heads per core
SCALE = 1.0 / float(np.sqrt(DK))

P = 128                  # SBUF/PSUM partitions
SC = 512                 # matmul moving-dim chunk
IB = 512                 # flash i-block
LOG2E = float(np.log2(np.e))
SCH_A = float(128.0 * SCALE * LOG2E)       # schraudolph slope
SCH_B = float(127.0 * 128.0 - 7.35)        # schraudolph bias (mean-one)


def build_nc(S=S, D=D, DH=DH, DK=DK, scale=SCALE, ib=IB):
    import concourse.bacc as bacc
    import concourse.mybir as mybir
    import concourse.tile as tile

    f32 = mybir.dt.float32
    bf16 = mybir.dt.bfloat16
    i16 = mybir.dt.int16
    Exp = mybir.ActivationFunctionType.Exp
    Ident = mybir.ActivationFunctionType.Identity
    Mult = mybir.AluOpType.mult
    Add = mybir.AluOpType.add
    cdt = bf16

    KT = D // P                    # contraction tiles for projections (8)
    NSC = S // SC                  # s chunks (4)
    HC = DH // P                   # head pairs (2)
    HPC = P // DK                  # heads per pair (2)
    JT = S // P                    # j tiles (16)
    NIB = S // ib                  # i blocks (4)
    NOUT = D // P                  # output row chunks (8)
    LAG = 2                        # AV trails scores by LAG j-steps

    nc = bacc.Bacc("TRN2", target_bir_lowering=False, debug=False)

    qT = nc.dram_tensor("qT", [D, S], cdt, kind="ExternalInput")
    kTd = nc.dram_tensor("kTd", [D, S], cdt, kind="ExternalInput")
    vT = nc.dram_tensor("vT", [D, S], cdt, kind="ExternalInput")
    wq = nc.dram_tensor("wq", [D, DH], cdt, kind="ExternalInput")
    wk = nc.dram_tensor("wk", [D, DH], cdt, kind="ExternalInput")
    wv = nc.dram_tensor("wv", [D, DH], cdt, kind="ExternalInput")
    wo = nc.dram_tensor("wo", [DH, D], cdt, kind="ExternalInput")
    bq = nc.dram_tensor("bq", [P, HC], f32, kind="ExternalInput")
    bk = nc.dram_tensor("bk", [P, HC], f32, kind="ExternalInput")
    bvb = nc.dram_tensor("bvb", [P, H_CORE, DK], f32, kind="ExternalInput")
    outT = nc.dram_tensor("outT", [D, S], cdt, kind="ExternalOutput")

    with tile.TileContext(nc) as tc:
        with (
            tc.tile_pool(name="const", bufs=1) as cpool,
            tc.tile_pool(name="pers", bufs=1) as pers,
            tc.tile_pool(name="stream", bufs=1) as stream,
            tc.tile_pool(name="psum", bufs=1, space="PSUM") as psum,
            tc.tile_pool(name="dscratch", bufs=1, space="DRAM") as dscratch,
        ):
            # ---- constants ----
            wq_sb = cpool.tile([P, KT, DH], cdt, name="wq_sb")
            wk_sb = cpool.tile([P, KT, DH], cdt, name="wk_sb")
            wv_sb = cpool.tile([P, KT, DH], cdt, name="wv_sb")
            wo_sb = cpool.tile([P, HC, D], cdt, name="wo_sb")
            bq_sb = cpool.tile([P, HC], f32, name="bq_sb")
            bk_sb = cpool.tile([P, HC], f32, name="bk_sb")
            bvb_sb = cpool.tile([P, H_CORE, DK], f32, name="bvb_sb")
            # weight/bias loads are interleaved with input-tensor loads
            # below so Q-proj can start as early as possible

            # ---- persistent activations (head-pair layout) ----
            # qt/kt pair c: rows 0-63 = head 2c (dk dims), rows 64-127 =
            # head 2c+1. v pair c: rows = j within tile, + ones column.
            qt = [pers.tile([P, S], cdt, name=f"qt{c}") for c in range(HC)]
            kt = [pers.tile([P, S], cdt, name=f"kt{c}") for c in range(HC)]
            v_c = [pers.tile([P, JT, HPC, DK + 1], cdt, name=f"v{c}")
                   for c in range(HC)]
            on_c = [pers.tile([P, S], cdt, name=f"on{c}") for c in range(HC)]

            for c in range(HC):
                nc.vector.memset(v_c[c][:, :, :, DK:DK + 1], 1.0)

            # ---- projections (inputs loaded as half-row 256KB DMAs) ----
            def load_tensor(src):
                bt = stream.tile([P, KT, S], cdt, tag="big_in", bufs=2,
                                 name=f"bi_{src.name}")
                for half in range(2):
                    hs = slice(half * (S // 2), (half + 1) * (S // 2))
                    for kti in range(KT):
                        nc.sync.dma_start(bt[:, kti, hs],
                                          src[kti * P:(kti + 1) * P, hs])
                return bt

            # DMA queue is FIFO: issue loads in consumption-priority order
            nc.sync.dma_start(wq_sb[:],
                              wq[:, :].rearrange("(ko p) n -> p ko n", p=P))
            nc.sync.dma_start(bq_sb[:], bq[:, :])
            qin = load_tensor(qT)
            nc.sync.dma_start(wk_sb[:],
                              wk[:, :].rearrange("(ko p) n -> p ko n", p=P))
            nc.sync.dma_start(bk_sb[:], bk[:, :])
            kin = load_tensor(kTd)
            nc.sync.dma_start(wv_sb[:],
                              wv[:, :].rearrange("(ko p) n -> p ko n", p=P))
            nc.sync.dma_start(bvb_sb[:], bvb[:, :, :])
            vin = load_tensor(vT)
            nc.sync.dma_start(wo_sb[:],
                              wo[:, :].rearrange("(c p) n -> p c n", p=P))

            def qk_proj(bt, w_sb, b_sb, dst):
                for si in range(NSC):
                    ps = psum.tile([P, 2 * SC], f32, tag="sc", bufs=2,
                                   name=f"ps_{dst[0].name}_{si}")
                    ssl = slice(si * SC, (si + 1) * SC)
                    for c in range(HC):
                        for kti in range(KT):
                            nc.tensor.matmul(
                                ps[:, c * SC:(c + 1) * SC],
                                lhsT=w_sb[:, kti, c * P:(c + 1) * P],
                                rhs=bt[:, kti, ssl],
                                start=(kti == 0), stop=(kti == KT - 1))
                    # evac + bias: head-pair chunk c goes straight to dst[c]
                    nc.vector.tensor_add(
                        dst[0][:, ssl], ps[:, 0:SC],
                        b_sb[:, 0:1].to_broadcast((P, SC)))
                    nc.scalar.activation(
                        dst[1][:, ssl], ps[:, SC:2 * SC], Ident,
                        bias=b_sb[:, 1:2], scale=1.0)

            qk_proj(qin, wq_sb, bq_sb, qt)
            qk_proj(kin, wk_sb, bk_sb, kt)

            # ---- V projection (natural [j, dh]) ----
            for si in range(NSC):
                for sub in range(SC // P):
                    jt_idx = si * (SC // P) + sub
                    ps = psum.tile([P, 2 * SC], f32, tag="sc", bufs=2,
                                   name=f"ps_v_{jt_idx}")
                    jsl = slice(si * SC + sub * P, si * SC + (sub + 1) * P)
                    for kti in range(KT):
                        nc.tensor.matmul(
                            ps[:, 0:DH],
                            lhsT=vin[:, kti, jsl],
                            rhs=wv_sb[:, kti, :],
                            start=(kti == 0), stop=(kti == KT - 1))
                    for c in range(HC):
                        src_ap = ps[:, c * P:(c + 1) * P].rearrange(
                            "p (h d) -> p h d", d=DK)
                        dst_ap = v_c[c][:, jt_idx, :, 0:DK]
                        bias_ap = bvb_sb[:, c * HPC:(c + 1) * HPC, :]
                        nc.vector.tensor_add(dst_ap, src_ap, bias_ap)

            # ---- attention (flash over j; head pairs on T0/T8) ----
            for c in range(HC):
                for ibx in range(NIB):
                    i0 = ibx * ib
                    isl = slice(i0, i0 + ib)
                    av = psum.tile([P, 2 * SC], f32, tag="av", bufs=2,
                                   name=f"av_{c}_{ibx}")
                    e_ts = {}
                    # batch 2 j-steps per group: 4 scores MMs (64x128 mode)
                    # then 4 AV MMs (128x128) -> fewer mode-switch drains
                    for jg in range(JT // 2 + 1):
                        for sub in range(2):
                            jt = 2 * jg + sub
                            if jt >= JT:
                                continue
                            sct = psum.tile([P, 2 * SC], f32, tag="sc",
                                            bufs=2, name=f"sc_{c}_{ibx}_{jt}")
                            jsl = slice(jt * P, (jt + 1) * P)
                            nc.tensor.matmul(
                                sct[:, 0:SC],
                                lhsT=kt[c][0:DK, jsl],
                                rhs=qt[c][0:DK, isl],
                                start=True, stop=True)
                            nc.tensor.matmul(
                                sct[:, SC:2 * SC],
                                lhsT=kt[c][DK:P, jsl],
                                rhs=qt[c][DK:P, isl],
                                start=True, stop=True)
                            et = stream.tile([P, 2 * SC], cdt, tag="e",
                                             bufs=4, name=f"e_{c}_{ibx}_{jt}")
                            if jt % 2 == 0 or jt == JT - 1 or DBG_EXP_ACT:
                                nc.scalar.activation(et[:], sct[:], Exp,
                                                     bias=0.0, scale=scale)
                            else:
                                nc.vector.tensor_scalar(
                                    et[:].bitcast(i16), sct[:],
                                    SCH_A, SCH_B, Mult, Add)
                            e_ts[jt] = et
                        for sub in range(2):
                            pj = 2 * (jg - 1) + sub
                            if pj < 0:
                                continue
                            et = e_ts.pop(pj)
                            st, sp = (pj == 0), (pj == JT - 1)
                            for h in range(HPC):
                                nc.tensor.matmul(
                                    av[0:DK + 1, h * SC:(h + 1) * SC],
                                    lhsT=v_c[c][:, pj, h, :],
                                    rhs=et[:, h * SC:(h + 1) * SC],
                                    start=st, stop=sp)
                    # ---- normalize (trails into next block) ----
                    # Z row -> SBUF (ACT), DRAM-reshape to [128, 8] for a
                    # cheap all-lane reciprocal, then broadcast-load.
                    zrow = stream.tile([P, 2 * SC], f32, tag="rz", bufs=2,
                                       name=f"rz_{c}_{ibx}")
                    nc.scalar.copy(zrow[DK:DK + 1, :], av[DK:DK + 1, :])
                    z_d = dscratch.tile([1, 2 * SC], f32, tag="zd", bufs=2,
                                        name=f"zd_{c}_{ibx}")
                    nc.sync.dma_start(z_d[:], zrow[DK:DK + 1, :])
                    zc = stream.tile([P, 2 * (2 * SC) // P], f32, tag="zc",
                                     bufs=2, name=f"zc_{c}_{ibx}")
                    zw = (2 * SC) // P
                    nc.sync.dma_start(
                        zc[:, 0:zw],
                        z_d[:, :].rearrange("o (p x) -> (o p) x", p=P))
                    nc.vector.reciprocal(zc[:, zw:2 * zw], zc[:, 0:zw])
                    rz_d = dscratch.tile([1, 2 * SC], f32, tag="rzd", bufs=2,
                                         name=f"rzd_{c}_{ibx}")
                    nc.sync.dma_start(
                        rz_d[:, :].rearrange("o (p x) -> (o p) x", p=P),
                        zc[:, zw:2 * zw])
                    rzb = stream.tile([DK, 2 * SC], f32, tag="rzb", bufs=2,
                                      name=f"rzb_{c}_{ibx}")
                    nc.sync.dma_start(
                        rzb[0:DK, :], rz_d[:, :].to_broadcast((DK, 2 * SC)))
                    nc.vector.tensor_mul(on_c[c][0:DK, isl],
                                         av[0:DK, 0:SC], rzb[0:DK, 0:SC])
                    stg = stream.tile([DK, SC], cdt, tag="stgB", bufs=2,
                                      name=f"stg_{c}_{ibx}")
                    nc.vector.tensor_mul(stg[0:DK, :],
                                         av[0:DK, SC:2 * SC],
                                         rzb[0:DK, SC:2 * SC])
                    nc.sync.dma_start(on_c[c][DK:P, isl], stg[0:DK, :])

            # ---- output projection (bias added on host) ----
            for n in range(NOUT):
                o_stg = stream.tile([P, S], cdt, tag="ostg", bufs=2,
                                    name=f"ostg_{n}")
                for i in range(NSC):
                    idx = n * NSC + i
                    pso = psum.tile([P, 2 * SC], f32,
                                    tag=("sc", "av")[idx % 2], bufs=2,
                                    name=f"ps_o_{n}_{i}")
                    for c in range(HC):
                        nc.tensor.matmul(
                            pso[:, 0:SC],
                            lhsT=wo_sb[:, c, n * P:(n + 1) * P],
                            rhs=on_c[c][:, i * SC:(i + 1) * SC],
                            start=(c == 0), stop=(c == HC - 1))
                    osl = slice(i * SC, (i + 1) * SC)
                    if idx % 2 == 0:
                        nc.scalar.copy(o_stg[:, osl], pso[:, 0:SC])
                    else:
                        nc.vector.tensor_copy(o_stg[:, osl], pso[:, 0:SC])
                nc.sync.dma_start(outT[n * P:(n + 1) * P, :], o_stg[:])

    nc.finalize()
    return nc


def make_in_maps(query, key, value, Wq, bq, Wk, bk, Wv, bv, Wo, bo):
    """Shard full inputs into the 8 per-core input dicts."""
    import ml_dtypes
    f = lambda a: np.ascontiguousarray(np.asarray(a, dtype=np.float32))
    HC = DH // P
    query, key, value = f(query), f(key), f(value)
    Wq, Wk, Wv, Wo = f(Wq), f(Wk), f(Wv), f(Wo)
    bq, bk, bv = f(bq), f(bk), f(bv)
    cvt = lambda a: np.ascontiguousarray(a.astype(ml_dtypes.bfloat16))
    in_maps = []
    for core in range(N_CORES):
        b, g = core // GROUPS, core % GROUPS
        sl = slice(g * DH, (g + 1) * DH)
        in_maps.append({
            "qT": cvt(query[b].T),
            "kTd": cvt(key[b].T),
            "vT": cvt(value[b].T),
            "wq": cvt(Wq[:, sl]),
            "wk": cvt(Wk[:, sl]),
            "wv": cvt(Wv[:, sl]),
            "wo": cvt(Wo[sl, :]),
            "bq": np.ascontiguousarray(bq[sl].reshape(HC, P).T),
            "bk": np.ascontiguousarray(bk[sl].reshape(HC, P).T),
            "bvb": np.ascontiguousarray(
                np.broadcast_to(bv[sl].reshape(H_CORE, DK)[None],
                                (P, H_CORE, DK))),
        })
    return in_maps


# test hooks (ignored by the harness)
TRACE = False
LAST_RESULT = None
DTYPE = "bf16"
_NC_CACHE = {}


def kernel(query, key, value, Wq, bq, Wk, bk, Wv, bv, Wo, bo):
    global LAST_RESULT
    from concourse.bass_utils import run_bass_kernel_spmd

    if "nc" not in _NC_CACHE:
        _NC_CACHE["nc"] = build_nc()
    nc = _NC_CACHE["nc"]

    in_maps = make_in_maps(query, key, value, Wq, bq, Wk, bk, Wv, bv, Wo, bo)
    kwargs = {}
    if TRACE:
        kwargs = dict(trace=True, trace_cores=[0])
    res = run_bass_kernel_spmd(nc, in_maps, core_ids=list(range(N_CORES)),
                               **kwargs)
    LAST_RESULT = res

    out = np.zeros((B, S, D), np.float32)
    for core in range(N_CORES):
        b = core // GROUPS
        out[b] += res.results[core]["outT"].T.astype(np.float32)
    out += np.asarray(bo, dtype=np.float32)
    return out


# revision 22
# speedup vs baseline: 1.5195x; 1.0745x over previous
"""Multi-head attention, tensor-parallel across 8 Trainium2 NeuronCores.

Sharding: core = (batch b, head-group g), g covering 4 heads (256 dh).
Within a core heads are processed as PAIRS using 64x128 PE row-tiling:
head A of a pair lives on SBUF partitions 0-63, head B on 64-127, so the
two K=64 scores matmuls run concurrently on array tiles T0/T8, and each
AV matmul's K=128 contraction is split into top/bot halves on T0/T8
accumulating into one PSUM bank via has_written.

exp is split between ScalarE (exact, even j-tiles) and VectorE (odd
j-tiles) using a round-to-nearest int16 Schraudolph: bf16bits(exp(x)) ~
round(x*A + B), verified exact-convert on HW; its mean ratio error is
calibrated out (softmax cancels any residual common-mode bias).

Z comes from a ones column appended to V (AV psum row 64). Normalize:
reciprocal_approx_fast on Z, DRAM-roundtrip broadcast, DVE mults; head
B's normalized block is staged and DMA'd to partitions 64-127 so the
output projection keeps K=128.

Host: shards inputs, sums the 4 head-group partials per batch, adds bo.
"""

import os
import numpy as np

DBG_AV_SINGLE = os.environ.get("DBG_AV_SINGLE", "0") == "1"
DBG_RECIP_PLAIN = os.environ.get("DBG_RECIP_PLAIN", "0") == "1"
DBG_EXP_ACT = os.environ.get("DBG_EXP_ACT", "0") == "1"

B, S, D, H = 2, 2048, 1024, 16
DK = D // H              # 64 head dim
N_CORES = 8
GROUPS = N_CORES // B    # 4 head-groups
DH = D // GROUPS         # 256 head-dims per core (4 heads)
H_CORE = DH // DK        # 4 heads per core
SCALE = 1.0 / float(np.sqrt(DK))

P = 128                  # SBUF/PSUM partitions
SC = 512                 # matmul moving-dim chunk
IB = 512                 # flash i-block
LOG2E = float(np.log2(np.e))
SCH_A = float(128.0 * SCALE * LOG2E)       # schraudolph slope
SCH_B = float(127.0 * 128.0 - 7.35)        # schraudolph bias (mean-one)


def build_nc(S=S, D=D, DH=DH, DK=DK, scale=SCALE, ib=IB):
    import concourse.bacc as bacc
    import concourse.mybir as mybir
    import concourse.tile as tile

    f32 = mybir.dt.float32
    bf16 = mybir.dt.bfloat16
    i16 = mybir.dt.int16
    Exp = mybir.ActivationFunctionType.Exp
    Ident = mybir.ActivationFunctionType.Identity
    Mult = mybir.AluOpType.mult
    Add = mybir.AluOpType.add
    cdt = bf16

    KT = D // P                    # contraction tiles for projections (8)
    NSC = S // SC                  # s chunks (4)
    HC = DH // P                   # head pairs (2)
    HPC = P // DK                  # heads per pair (2)
    JT = S // P                    # j tiles (16)
    NIB = S // ib                  # i blocks (4)
    NOUT = D // P                  # output row chunks (8)
    LAG = 2                        # AV trails scores by LAG j-steps

    nc = bacc.Bacc("TRN2", target_bir_lowering=False, debug=False)

    qT = nc.dram_tensor("qT", [D, S], cdt, kind="ExternalInput")
    kTd = nc.dram_tensor("kTd", [D, S], cdt, kind="ExternalInput")
    vT = nc.dram_tensor("vT", [D, S], cdt, kind="ExternalInput")
    wq = nc.dram_tensor("wq", [D, DH], cdt, kind="ExternalInput")
    wk = nc.dram_tensor("wk", [D, DH], cdt, kind="ExternalInput")
    wv = nc.dram_tensor("wv", [D, DH], cdt, kind="ExternalInput")
    wo = nc.dram_tensor("wo", [DH, D], cdt, kind="ExternalInput")
    bq = nc.dram_tensor("bq", [P, HC], f32, kind="ExternalInput")
    bk = nc.dram_tensor("bk", [P, HC], f32, kind="ExternalInput")
    bvb = nc.dram_tensor("bvb", [P, H_CORE, DK], f32, kind="ExternalInput")
    outT = nc.dram_tensor("outT", [D, S], cdt, kind="ExternalOutput")

    with tile.TileContext(nc) as tc:
        with (
            tc.tile_pool(name="const", bufs=1) as cpool,
            tc.tile_pool(name="pers", bufs=1) as pers,
            tc.tile_pool(name="stream", bufs=1) as stream,
            tc.tile_pool(name="psum", bufs=1, space="PSUM") as psum,
            tc.tile_pool(name="dscratch", bufs=1, space="DRAM") as dscratch,
        ):
            # ---- constants ----
            wq_sb = cpool.tile([P, KT, DH], cdt, name="wq_sb")
            wk_sb = cpool.tile([P, KT, DH], cdt, name="wk_sb")
            wv_sb = cpool.tile([P, KT, DH], cdt, name="wv_sb")
            wo_sb = cpool.tile([P, HC, D], cdt, name="wo_sb")
            bq_sb = cpool.tile([P, HC], f32, name="bq_sb")
            bk_sb = cpool.tile([P, HC], f32, name="bk_sb")
            bvb_sb = cpool.tile([P, H_CORE, DK], f32, name="bvb_sb")
            # weight/bias loads are interleaved with input-tensor loads
            # below so Q-proj can start as early as possible

            # ---- persistent activations (head-pair layout) ----
            # qt/kt pair c: rows 0-63 = head 2c (dk dims), rows 64-127 =
            # head 2c+1. v pair c: rows = j within tile, + ones column.
            qt = [pers.tile([P, S], cdt, name=f"qt{c}") for c in range(HC)]
            kt = [pers.tile([P, S], cdt, name=f"kt{c}") for c in range(HC)]
            v_c = [pers.tile([P, JT, HPC, DK + 1], cdt, name=f"v{c}")
                   for c in range(HC)]
            on_c = [pers.tile([P, S], cdt, name=f"on{c}") for c in range(HC)]

            for c in range(HC):
                nc.vector.memset(v_c[c][:, :, :, DK:DK + 1], 1.0)

            # ---- projections (inputs loaded as half-row 256KB DMAs) ----
            def load_tensor(src):
                bt = stream.tile([P, KT, S], cdt, tag="big_in", bufs=2,
                                 name=f"bi_{src.name}")
                for half in range(2):
                    hs = slice(half * (S // 2), (half + 1) * (S // 2))
                    for kti in range(KT):
                        nc.sync.dma_start(bt[:, kti, hs],
                                          src[kti * P:(kti + 1) * P, hs])
                return bt

            # DMA queue is FIFO: issue loads in consumption-priority order
            nc.sync.dma_start(wq_sb[:],
                              wq[:, :].rearrange("(ko p) n -> p ko n", p=P))
            nc.sync.dma_start(bq_sb[:], bq[:, :])
            qin = load_tensor(qT)
            nc.sync.dma_start(wk_sb[:],
                              wk[:, :].rearrange("(ko p) n -> p ko n", p=P))
            nc.sync.dma_start(bk_sb[:], bk[:, :])
            kin = load_tensor(kTd)
            nc.sync.dma_start(wv_sb[:],
                              wv[:, :].rearrange("(ko p) n -> p ko n", p=P))
            nc.sync.dma_start(bvb_sb[:], bvb[:, :, :])
            vin = load_tensor(vT)
            nc.sync.dma_start(wo_sb[:],
                              wo[:, :].rearrange("(c p) n -> p c n", p=P))

            def qk_proj(bt, w_sb, b_sb, dst):
                for si in range(NSC):
                    ps = psum.tile([P, 2 * SC], f32, tag="sc", bufs=3,
                                   name=f"ps_{dst[0].name}_{si}")
                    ssl = slice(si * SC, (si + 1) * SC)
                    for c in range(HC):
                        for kti in range(KT):
                            nc.tensor.matmul(
                                ps[:, c * SC:(c + 1) * SC],
                                lhsT=w_sb[:, kti, c * P:(c + 1) * P],
                                rhs=bt[:, kti, ssl],
                                start=(kti == 0), stop=(kti == KT - 1))
                    # evac + bias: head-pair chunk c goes straight to dst[c]
                    nc.vector.tensor_add(
                        dst[0][:, ssl], ps[:, 0:SC],
                        b_sb[:, 0:1].to_broadcast((P, SC)))
                    nc.scalar.activation(
                        dst[1][:, ssl], ps[:, SC:2 * SC], Ident,
                        bias=b_sb[:, 1:2], scale=1.0)

            qk_proj(qin, wq_sb, bq_sb, qt)
            qk_proj(kin, wk_sb, bk_sb, kt)

            # ---- V projection (natural [j, dh]) ----
            for si in range(NSC):
                for sub in range(SC // P):
                    jt_idx = si * (SC // P) + sub
                    ps = psum.tile([P, 2 * SC], f32, tag="sc", bufs=3,
                                   name=f"ps_v_{jt_idx}")
                    jsl = slice(si * SC + sub * P, si * SC + (sub + 1) * P)
                    for kti in range(KT):
                        nc.tensor.matmul(
                            ps[:, 0:DH],
                            lhsT=vin[:, kti, jsl],
                            rhs=wv_sb[:, kti, :],
                            start=(kti == 0), stop=(kti == KT - 1))
                    for c in range(HC):
                        src_ap = ps[:, c * P:(c + 1) * P].rearrange(
                            "p (h d) -> p h d", d=DK)
                        dst_ap = v_c[c][:, jt_idx, :, 0:DK]
                        bias_ap = bvb_sb[:, c * HPC:(c + 1) * HPC, :]
                        nc.vector.tensor_add(dst_ap, src_ap, bias_ap)

            # ---- attention (flash over j; head pairs on T0/T8) ----
            for c in range(HC):
                for ibx in range(NIB):
                    i0 = ibx * ib
                    isl = slice(i0, i0 + ib)
                    av = psum.tile([P, 2 * SC], f32, tag="av", bufs=1,
                                   name=f"av_{c}_{ibx}")
                    e_ts = {}
                    # batch 2 j-steps per group: 4 scores MMs (64x128 mode)
                    # then 4 AV MMs (128x128) -> fewer mode-switch drains
                    for jg in range(JT // 2 + 1):
                        for sub in range(2):
                            jt = 2 * jg + sub
                            if jt >= JT:
                                continue
                            sct = psum.tile([P, 2 * SC], f32, tag="sc",
                                            bufs=3, name=f"sc_{c}_{ibx}_{jt}")
                            jsl = slice(jt * P, (jt + 1) * P)
                            nc.tensor.matmul(
                                sct[:, 0:SC],
                                lhsT=kt[c][0:DK, jsl],
                                rhs=qt[c][0:DK, isl],
                                start=True, stop=True)
                            nc.tensor.matmul(
                                sct[:, SC:2 * SC],
                                lhsT=kt[c][DK:P, jsl],
                                rhs=qt[c][DK:P, isl],
                                start=True, stop=True)
                            et = stream.tile([P, 2 * SC], cdt, tag="e",
                                             bufs=5, name=f"e_{c}_{ibx}_{jt}")
                            if jt % 2 == 0 or jt == JT - 1 or DBG_EXP_ACT:
                                nc.scalar.activation(et[:], sct[:], Exp,
                                                     bias=0.0, scale=scale)
                            else:
                                nc.vector.tensor_scalar(
                                    et[:].bitcast(i16), sct[:],
                                    SCH_A, SCH_B, Mult, Add)
                            e_ts[jt] = et
                        for sub in range(2):
                            pj = 2 * (jg - 1) + sub
                            if pj < 0:
                                continue
                            et = e_ts.pop(pj)
                            st, sp = (pj == 0), (pj == JT - 1)
                            for h in range(HPC):
                                nc.tensor.matmul(
                                    av[0:DK + 1, h * SC:(h + 1) * SC],
                                    lhsT=v_c[c][:, pj, h, :],
                                    rhs=et[:, h * SC:(h + 1) * SC],
                                    start=st, stop=sp)
                    # ---- normalize (trails into next block) ----
                    # Evacuate av to SBUF immediately (frees the psum bank;
                    # av bufs=1). Z row DMAs through DRAM reshaped [128, 8]
                    # for a cheap all-lane reciprocal; mults run on GPSIMD.
                    av_sb = stream.tile([P, 2 * SC], f32, tag="avsb", bufs=2,
                                        name=f"avsb_{c}_{ibx}")
                    nc.scalar.copy(av_sb[0:DK + 1, :], av[0:DK + 1, :])
                    z_d = dscratch.tile([1, 2 * SC], f32, tag="zd", bufs=2,
                                        name=f"zd_{c}_{ibx}")
                    nc.sync.dma_start(z_d[:], av_sb[DK:DK + 1, :])
                    zc = stream.tile([P, 2 * (2 * SC) // P], f32, tag="zc",
                                     bufs=2, name=f"zc_{c}_{ibx}")
                    zw = (2 * SC) // P
                    nc.sync.dma_start(
                        zc[:, 0:zw],
                        z_d[:, :].rearrange("o (p x) -> (o p) x", p=P))
                    nc.vector.reciprocal(zc[:, zw:2 * zw], zc[:, 0:zw])
                    rz_d = dscratch.tile([1, 2 * SC], f32, tag="rzd", bufs=2,
                                         name=f"rzd_{c}_{ibx}")
                    nc.sync.dma_start(
                        rz_d[:, :].rearrange("o (p x) -> (o p) x", p=P),
                        zc[:, zw:2 * zw])
                    rzb = stream.tile([DK, 2 * SC], f32, tag="rzb", bufs=2,
                                      name=f"rzb_{c}_{ibx}")
                    nc.sync.dma_start(
                        rzb[0:DK, :], rz_d[:, :].to_broadcast((DK, 2 * SC)))
                    nc.gpsimd.tensor_mul(on_c[c][0:DK, isl],
                                         av_sb[0:DK, 0:SC], rzb[0:DK, 0:SC])
                    stg = stream.tile([DK, SC], cdt, tag="stgB", bufs=2,
                                      name=f"stg_{c}_{ibx}")
                    nc.gpsimd.tensor_mul(stg[0:DK, :],
                                         av_sb[0:DK, SC:2 * SC],
                                         rzb[0:DK, SC:2 * SC])
                    nc.sync.dma_start(on_c[c][DK:P, isl], stg[0:DK, :])

            # ---- output projection (bias added on host) ----
            # i-outer so only the last i-chunk waits on the final normalize;
            # per-n staging halves so outT stores are 256KB each.
            o_stgs = [stream.tile([P, 2 * SC], cdt, tag="ostg", bufs=NOUT,
                                  name=f"ostg_{n}") for n in range(NOUT)]
            for i in range(NSC):
                for n in range(NOUT):
                    idx = i * NOUT + n
                    pso = psum.tile([P, 2 * SC], f32, tag="sc", bufs=3,
                                    name=f"ps_o_{n}_{i}")
                    for c in range(HC):
                        nc.tensor.matmul(
                            pso[:, 0:SC],
                            lhsT=wo_sb[:, c, n * P:(n + 1) * P],
                            rhs=on_c[c][:, i * SC:(i + 1) * SC],
                            start=(c == 0), stop=(c == HC - 1))
                    osl = slice((i % 2) * SC, (i % 2 + 1) * SC)
                    if idx % 2 == 0:
                        nc.scalar.copy(o_stgs[n][:, osl], pso[:, 0:SC])
                    else:
                        nc.vector.tensor_copy(o_stgs[n][:, osl], pso[:, 0:SC])
                    if i % 2 == 1:
                        nc.sync.dma_start(
                            outT[n * P:(n + 1) * P,
                                 (i - 1) * SC:(i + 1) * SC],
                            o_stgs[n][:])

    nc.finalize()
    return nc


def make_in_maps(query, key, value, Wq, bq, Wk, bk, Wv, bv, Wo, bo):
    """Shard full inputs into the 8 per-core input dicts."""
    import ml_dtypes
    f = lambda a: np.ascontiguousarray(np.asarray(a, dtype=np.float32))
    HC = DH // P
    query, key, value = f(query), f(key), f(value)
    Wq, Wk, Wv, Wo = f(Wq), f(Wk), f(Wv), f(Wo)
    bq, bk, bv = f(bq), f(bk), f(bv)
    cvt = lambda a: np.ascontiguousarray(a.astype(ml_dtypes.bfloat16))
    in_maps = []
    for core in range(N_CORES):
        b, g = core // GROUPS, core % GROUPS
        sl = slice(g * DH, (g + 1) * DH)
        in_maps.append({
            "qT": cvt(query[b].T),
            "kTd": cvt(key[b].T),
            "vT": cvt(value[b].T),
            "wq": cvt(Wq[:, sl]),
            "wk": cvt(Wk[:, sl]),
            "wv": cvt(Wv[:, sl]),
            "wo": cvt(Wo[sl, :]),
            "bq": np.ascontiguousarray(bq[sl].reshape(HC, P).T),
            "bk": np.ascontiguousarray(bk[sl].reshape(HC, P).T),
            "bvb": np.ascontiguousarray(
                np.broadcast_to(bv[sl].reshape(H_CORE, DK)[None],
                                (P, H_CORE, DK))),
        })
    return in_maps


# test hooks (ignored by the harness)
TRACE = False
LAST_RESULT = None
DTYPE = "bf16"
_NC_CACHE = {}


def kernel(query, key, value, Wq, bq, Wk, bk, Wv, bv, Wo, bo):
    global LAST_RESULT
    from concourse.bass_utils import run_bass_kernel_spmd

    if "nc" not in _NC_CACHE:
        _NC_CACHE["nc"] = build_nc()
    nc = _NC_CACHE["nc"]

    in_maps = make_in_maps(query, key, value, Wq, bq, Wk, bk, Wv, bv, Wo, bo)
    kwargs = {}
    if TRACE:
        kwargs = dict(trace=True, trace_cores=[0])
    res = run_bass_kernel_spmd(nc, in_maps, core_ids=list(range(N_CORES)),
                               **kwargs)
    LAST_RESULT = res

    out = np.zeros((B, S, D), np.float32)
    for core in range(N_CORES):
        b = core // GROUPS
        out[b] += res.results[core]["outT"].T.astype(np.float32)
    out += np.asarray(bo, dtype=np.float32)
    return out
